# revision 2
# baseline (speedup 1.0000x reference)
"""FIRE self-attention TRN2 kernel.

Full inputs -> full output. Sharding: one attention head per NeuronCore
(8 heads / 8 cores, tensor parallel). Each core computes its head's FIRE
bias, QK^T logits, softmax, AV, and its head's slice of the output
projection; the host sums the 8 partial projections.

Key algorithmic points:
  * raw[i,j] = log1p(c*relu(i-j)) / log1p(c*max(L, i+1)) is always in
    [0, 1] (numerator <= denominator since i-j <= max(L, i+1)), and the
    per-head scalar MLP bias = f_theta(raw) is a smooth 1-D function, so
    it is evaluated as a cubic polynomial fitted on [0, 1] (fit error
    ~1e-7, far below fp32 matmul noise).
  * Everything runs in a transposed layout: logits^T[j, i] so that the
    softmax normalization axis (j) lands on PSUM partitions, attn^T is
    directly the stationary operand of the AV matmul, and a ones-column
    appended to V yields the softmax row sums for free.
  * The bias enters PSUM through an identity matmul so QK^T accumulates
    on top of it; the causal mask is -30000 added to the strict upper
    triangle (exp -> 0).
  * Softmax normalization (1/rowsum) is folded into the PSUM->SBUF copy
    of the output projection as a per-partition tensor_scalar multiply.
"""

import math
from contextlib import ExitStack

import numpy as np

import concourse.bacc as bacc
import concourse.bass as bass
import concourse.mybir as mybir
import concourse.tile as tile
from concourse.bass_utils import run_bass_kernel_spmd

F32 = mybir.dt.float32
AF = mybir.ActivationFunctionType
ALU = mybir.AluOpType

B, S, D, H, KD, HID = 8, 1024, 512, 8, 64, 32
P = 128
NJC = S // P  # 8 key-blocks of 128
NCORES = 8
MASK_NEG = -30000.0

# consts tensor column layout: cvec | coef(4) | maskd(128) | ident(128) | t0(S) | rdb(S)
_C_CVEC = 0
_C_COEF = 1
_C_MASK = 5
_C_IDENT = 133
_C_T0 = 261
_C_RDB = 261 + S
_C_TOT = 261 + 2 * S


def _build_kernel(ctx: ExitStack, tc: "tile.TileContext", dr):
    nc = tc.nc

    pconst = ctx.enter_context(tc.tile_pool(name="const", bufs=1))
    pbias = ctx.enter_context(tc.tile_pool(name="bias", bufs=1))
    ptmp = ctx.enter_context(tc.tile_pool(name="tmp", bufs=1))
    psrc = ctx.enter_context(tc.tile_pool(name="src", bufs=2))
    pqk = ctx.enter_context(tc.tile_pool(name="qk", bufs=2))
    pvp = ctx.enter_context(tc.tile_pool(name="vp", bufs=2))
    pattn = ctx.enter_context(tc.tile_pool(name="attn", bufs=3))
    posb = ctx.enter_context(tc.tile_pool(name="osb", bufs=2))
    prs = ctx.enter_context(tc.tile_pool(name="rs", bufs=2))
    pout = ctx.enter_context(tc.tile_pool(name="outst", bufs=3))

    ps_log = ctx.enter_context(
        tc.tile_pool(name="pslog", bufs=2, space=bass.MemorySpace.PSUM)
    )
    ps_oT = ctx.enter_context(
        tc.tile_pool(name="psoT", bufs=1, space=bass.MemorySpace.PSUM)
    )
    ps_misc = ctx.enter_context(
        tc.tile_pool(name="psmisc", bufs=2, space=bass.MemorySpace.PSUM)
    )

    # ---- constants / weights into SBUF
    consts = pconst.tile([P, _C_TOT], F32)
    nc.sync.dma_start(consts[:], dr["consts"][:])
    cvec = consts[:, _C_CVEC : _C_CVEC + 1]
    coef = consts[:, _C_COEF : _C_COEF + 4]
    maskd = consts[:, _C_MASK : _C_MASK + P]
    ident = consts[:, _C_IDENT : _C_IDENT + P]
    t0 = consts[:, _C_T0 : _C_T0 + S]
    rdb = consts[:, _C_RDB : _C_RDB + S]

    wqkv = pconst.tile([P, 3, 4, KD], F32)  # packed (Wq/8, Wk, Wv) lhsT chunks
    nc.sync.dma_start(wqkv[:], dr["wqkv"][:])
    wo = pconst.tile([KD, D], F32)
    nc.sync.dma_start(wo[:], dr["wo"][:])

    # ---- FIRE bias (transposed): biasT[:, jc, n] = bias^T[128*jc + p, 128*jc + n]
    biasT = pbias.tile([P, NJC, S], F32)
    for jc in range(NJC):
        W = S - P * jc
        num = ptmp.tile([P, S], F32, tag="tA")
        nc.vector.tensor_scalar(num[:, :W], t0[:, :W], 0.0, cvec, ALU.max, ALU.mult)
        lnv = ptmp.tile([P, S], F32, tag="tB")
        nc.scalar.activation(lnv[:, :W], num[:, :W], AF.Ln, bias=1.0, scale=1.0)
        r = ptmp.tile([P, S], F32, tag="tC")
        nc.vector.tensor_tensor(r[:, :W], lnv[:, :W], rdb[:, P * jc : P * jc + W], ALU.mult)
        q1 = ptmp.tile([P, S], F32, tag="tA")
        nc.vector.tensor_scalar(
            q1[:, :W], r[:, :W], coef[:, 3:4], coef[:, 1:2], ALU.mult, ALU.add
        )
        q0 = ptmp.tile([P, S], F32, tag="tB")
        nc.vector.tensor_scalar(
            q0[:, :W], r[:, :W], coef[:, 2:3], coef[:, 0:1], ALU.mult, ALU.add
        )
        r2 = ptmp.tile([P, S], F32, tag="tD")
        nc.vector.tensor_tensor(r2[:, :W], r[:, :W], r[:, :W], ALU.mult)
        t = ptmp.tile([P, S], F32, tag="tC")
        nc.vector.tensor_tensor(t[:, :W], r2[:, :W], q1[:, :W], ALU.mult)
        nc.vector.tensor_tensor(biasT[:, jc, :W], t[:, :W], q0[:, :W], ALU.add)
        # causal mask on the diagonal 128-block (j > i -> -30000)
        nc.vector.tensor_tensor(
            biasT[:, jc, 0:P], biasT[:, jc, 0:P], maskd, ALU.add
        )

    # ---- attention, one batch at a time
    for b in range(B):
        st = psrc.tile([P, 4, S], F32)
        for c in range(4):
            nc.sync.dma_start(st[:, c, :], dr["srcT"][b, P * c : P * (c + 1), :])

        # q^T, k^T, v^T : [KD, S] = W[KD, D] @ src^T (contraction over d-chunks)
        qkvT = []
        for w_i, tg in ((0, "qT"), (1, "kT"), (2, "vT")):
            dst = pqk.tile([KD, S], F32, tag=tg)
            for half in range(2):
                pp = ps_misc.tile([P, 512], F32, tag="pm")
                for c in range(4):
                    nc.tensor.matmul(
                        pp[:KD, :],
                        wqkv[:, w_i, c, :],
                        st[:, c, 512 * half : 512 * (half + 1)],
                        start=(c == 0),
                        stop=(c == 3),
                    )
                nc.vector.tensor_copy(dst[:, 512 * half : 512 * (half + 1)], pp[:KD, :])
            qkvT.append(dst)
        qT, kT, vT = qkvT

        # v' = [v natural | ones column] per key-block, via PE transpose
        vp = pvp.tile([P, NJC, KD + 1], F32)
        for jc in range(NJC):
            pt = ps_misc.tile([P, 512], F32, tag="pm")
            nc.tensor.transpose(
                pt[:, :KD], vT[:, P * jc : P * (jc + 1)], ident[:KD, :KD]
            )
            nc.vector.tensor_copy(vp[:, jc, :KD], pt[:, :KD])
            nc.vector.memset(vp[:, jc, KD : KD + 1], 1.0)

        # logits^T -> exp -> AV (triangular: i-window [128*jc, S))
        oT = ps_oT.tile([KD + 1, S], F32)
        for jc in range(NJC):
            W = S - P * jc
            lg = ps_log.tile([P, S], F32)
            for n0 in range(0, W, 512):
                nn = min(512, W - n0)
                nc.tensor.matmul(
                    lg[:, n0 : n0 + nn],
                    ident,
                    biasT[:, jc, n0 : n0 + nn],
                    start=True,
                    stop=False,
                    skip_group_check=True,
                )
                nc.tensor.matmul(
                    lg[:, n0 : n0 + nn],
                    kT[:, P * jc : P * (jc + 1)],
                    qT[:, P * jc + n0 : P * jc + n0 + nn],
                    start=False,
                    stop=True,
                    skip_group_check=True,
                )
            at = pattn.tile([P, S], F32)
            nc.scalar.activation(at[:, :W], lg[:, :W], AF.Exp)
            # accumulate into oT output chunks [0,512) and [512,1024)
            for oc in (0, 512):
                lo = max(oc, P * jc)
                hi = oc + 512
                if lo >= hi:
                    continue
                n0 = lo - P * jc
                nc.tensor.matmul(
                    oT[:, lo:hi],
                    vp[:, jc, :],
                    at[:, n0 : n0 + (hi - lo)],
                    start=(jc == 0),
                    stop=(jc == NJC - 1 or (oc == 0 and jc == 3)),
                    skip_group_check=True,
                )

        # reciprocal row sums, transposed to [tok-on-partition, tile] via tiny matmuls
        rs_row = prs.tile([1, S], F32, tag="rsrow")
        nc.vector.reciprocal(rs_row[:], oT[KD : KD + 1, :])
        rsT_ps = ps_misc.tile([P, 512], F32, tag="pm")
        for ti in range(NJC):
            nc.tensor.matmul(
                rsT_ps[:, ti : ti + 1],
                rs_row[0:1, P * ti : P * (ti + 1)],
                ident[0:1, 0:1],
                start=True,
                stop=True,
                skip_group_check=True,
            )
        rsT = prs.tile([P, NJC], F32, tag="rsT")
        nc.vector.tensor_copy(rsT[:], rsT_ps[:, :NJC])

        osb = posb.tile([KD, S], F32)
        nc.vector.tensor_copy(osb[:, 0:512], oT[:KD, 0:512])
        nc.vector.tensor_copy(osb[:, 512:S], oT[:KD, 512:S])

        # partial out = (o * rs) @ Wo_h^T ; rs folded into the PSUM->SBUF copy
        for ti in range(NJC):
            po = ps_misc.tile([P, 512], F32, tag="pm")
            nc.tensor.matmul(
                po[:], osb[:, P * ti : P * (ti + 1)], wo[:], start=True, stop=True
            )
            ob = pout.tile([P, D], F32)
            nc.vector.tensor_scalar(
                ob[:], po[:], rsT[:, ti : ti + 1], None, ALU.mult
            )
            nc.sync.dma_start(dr["out"][b, P * ti : P * (ti + 1), :], ob[:])


_NC_CACHE = {}


def _get_nc():
    if "nc" in _NC_CACHE:
        return _NC_CACHE["nc"]
    nc = bacc.Bacc("TRN2", target_bir_lowering=False, debug=False, num_devices=NCORES)
    dr = {
        "srcT": nc.dram_tensor("srcT", [B, D, S], F32, kind="ExternalInput"),
        "wqkv": nc.dram_tensor("wqkv", [P, 3, 4, KD], F32, kind="ExternalInput"),
        "wo": nc.dram_tensor("wo", [KD, D], F32, kind="ExternalInput"),
        "consts": nc.dram_tensor("consts", [P, _C_TOT], F32, kind="ExternalInput"),
        "out": nc.dram_tensor("out", [B, S, D], F32, kind="ExternalOutput"),
    }
    with tile.TileContext(nc) as tc:
        with ExitStack() as ctx:
            _build_kernel(ctx, tc, dr)
    nc.compile()
    _NC_CACHE["nc"] = nc
    return nc


_erf = np.frompyfunc(math.erf, 1, 1)


def _gelu64(x):
    return 0.5 * x * (1.0 + _erf(x).astype(np.float64))


def _host_prep(inputs):
    """Per-core input tensors (one head per core)."""
    src = np.ascontiguousarray(inputs["src"], dtype=np.float32)
    srcT = np.ascontiguousarray(src.transpose(0, 2, 1))  # [B, D, S]

    t0 = (
        np.arange(S, dtype=np.float32)[None, :]
        - np.arange(P, dtype=np.float32)[:, None]
    )
    maskd = np.where(
        np.arange(P)[:, None] > np.arange(P)[None, :], np.float32(MASK_NEG), 0.0
    ).astype(np.float32)
    identity = np.eye(P, dtype=np.float32)

    grid = np.linspace(0.0, 1.0, 4097)
    in_maps = []
    for h in range(H):
        c = float(np.logaddexp(0.0, np.float64(inputs["c_raw"][h])))
        L = float(inputs["L"][h])
        i = np.arange(S, dtype=np.float64)
        rd = (1.0 / np.log1p(c * np.maximum(L, i + 1.0))).astype(np.float32)
        rdb = np.broadcast_to(rd[None, :], (P, S))

        w1 = inputs["w1"][h].astype(np.float64)
        b1 = inputs["b1"][h].astype(np.float64)
        W2 = inputs["W2"][h].astype(np.float64)
        b2 = inputs["b2"][h].astype(np.float64)
        w3 = inputs["w3"][h].astype(np.float64)
        b3 = float(inputs["b3"][h])
        h1 = _gelu64(grid[:, None] * w1[None, :] + b1[None, :]).astype(np.float64)
        h2 = _gelu64(h1 @ W2.T + b2[None, :]).astype(np.float64)
        vals = h2 @ w3 + b3
        c3, c2, c1, c0 = np.polyfit(grid, vals, 3)

        consts = np.zeros((P, _C_TOT), np.float32)
        consts[:, _C_CVEC] = c
        consts[:, _C_COEF : _C_COEF + 4] = np.float32([c0, c1, c2, c3])
        consts[:, _C_MASK : _C_MASK + P] = maskd
        consts[:, _C_IDENT : _C_IDENT + P] = identity
        consts[:, _C_T0 : _C_T0 + S] = t0
        consts[:, _C_RDB : _C_RDB + S] = rdb

        # lhsT chunks: wqkv[p, w, ch, kd] = W[kd, 128*ch + p]  (Wq scaled by 1/8)
        wqkv = np.zeros((P, 3, 4, KD), np.float32)
        for w_i, (w_arr, scale) in enumerate(
            ((inputs["Wq"][h], 1.0 / 8.0), (inputs["Wk"][h], 1.0), (inputs["Wv"][h], 1.0))
        ):
            wt = (w_arr.astype(np.float64) * scale).astype(np.float32)  # [KD, D]
            wqkv[:, w_i, :, :] = wt.T.reshape(4, P, KD).transpose(1, 0, 2)

        wo = np.ascontiguousarray(
            inputs["Wo"][:, h * KD : (h + 1) * KD].T, dtype=np.float32
        )  # [KD, D]

        in_maps.append(
            {
                "srcT": srcT,
                "wqkv": wqkv,
                "wo": wo,
                "consts": consts,
            }
        )
    return in_maps


def run_on_device(inputs, **spmd_kwargs):
    """Compile (cached) + run; returns (per-core result dicts, BassKernelResults)."""
    nc = _get_nc()
    in_maps = _host_prep(inputs)
    res = run_bass_kernel_spmd(nc, in_maps, list(range(NCORES)), **spmd_kwargs)
    return res


def kernel(**inputs) -> np.ndarray:
    inputs = {k: np.asarray(v) for k, v in inputs.items()}
    res = run_on_device(inputs)
    out = np.zeros((B, S, D), np.float32)
    for h in range(H):
        out += res.results[h]["out"]
    return out


# revision 5
# speedup vs baseline: 1.1234x; 1.1234x over previous
"""FIRE self-attention TRN2 kernel.

Full inputs -> full output. Sharding: one attention head per NeuronCore
(8 heads / 8 cores, tensor parallel). Each core computes its head's FIRE
bias, QK^T logits, softmax, AV, and its head's slice of the output
projection; the host sums the 8 partial projections.

Key algorithmic points:
  * raw[i,j] = log1p(c*relu(i-j)) / log1p(c*max(L, i+1)) is always in
    [0, 1] (numerator <= denominator since i-j <= max(L, i+1)), and the
    per-head scalar MLP bias = f_theta(raw) is a smooth 1-D function, so
    it is evaluated as a cubic polynomial fitted on [0, 1] (fit error
    ~1e-7, far below fp32 matmul noise).
  * Everything runs in a transposed layout: logits^T[j, i] so that the
    softmax normalization axis (j) lands on PSUM partitions, attn^T is
    directly the stationary operand of the AV matmul, and a ones-column
    appended to V yields the softmax row sums for free.
  * The bias enters PSUM through an identity matmul so QK^T accumulates
    on top of it; the causal mask is -30000 added to the strict upper
    triangle (exp -> 0).
  * Softmax normalization (1/rowsum) is folded into the PSUM->SBUF copy
    of the output projection as a per-partition tensor_scalar multiply.
"""

import math
from contextlib import ExitStack

import numpy as np

import concourse.bacc as bacc
import concourse.bass as bass
import concourse.mybir as mybir
import concourse.tile as tile
from concourse.bass_utils import run_bass_kernel_spmd

F32 = mybir.dt.float32
AF = mybir.ActivationFunctionType
ALU = mybir.AluOpType

B, S, D, H, KD, HID = 8, 1024, 512, 8, 64, 32
P = 128
NJC = S // P  # 8 key-blocks of 128
NCORES = 8
MASK_NEG = -30000.0

# consts tensor column layout: cvec | coef(4) | maskd(128) | ident(128) | t0(S) | rdb(S)
_C_CVEC = 0
_C_COEF = 1
_C_MASK = 5
_C_IDENT = 133
_C_T0 = 261
_C_RDB = 261 + S
_C_TOT = 261 + 2 * S


def _build_kernel(ctx: ExitStack, tc: "tile.TileContext", dr):
    nc = tc.nc

    pconst = ctx.enter_context(tc.tile_pool(name="const", bufs=1))
    pbias = ctx.enter_context(tc.tile_pool(name="bias", bufs=1))
    ptmp = ctx.enter_context(tc.tile_pool(name="tmp", bufs=1))
    psrc = ctx.enter_context(tc.tile_pool(name="src", bufs=2))
    pqk = ctx.enter_context(tc.tile_pool(name="qk", bufs=2))
    pvp = ctx.enter_context(tc.tile_pool(name="vp", bufs=2))
    pattn = ctx.enter_context(tc.tile_pool(name="attn", bufs=3))
    posb = ctx.enter_context(tc.tile_pool(name="osb", bufs=2))
    prs = ctx.enter_context(tc.tile_pool(name="rs", bufs=2))
    pout = ctx.enter_context(tc.tile_pool(name="outst", bufs=3))

    # 4 PSUM pools x 2 banks = 8 banks, all double-buffered so batch b+1's
    # projections never wait on batch b's output-projection tail.
    ps_log = ctx.enter_context(
        tc.tile_pool(name="pslog", bufs=2, space=bass.MemorySpace.PSUM)
    )
    ps_oT = ctx.enter_context(
        tc.tile_pool(name="psoT", bufs=1, space=bass.MemorySpace.PSUM)
    )
    ps_proj = ctx.enter_context(
        tc.tile_pool(name="psproj", bufs=2, space=bass.MemorySpace.PSUM)
    )
    ps_wo = ctx.enter_context(
        tc.tile_pool(name="pswo", bufs=2, space=bass.MemorySpace.PSUM)
    )

    # ---- constants / weights into SBUF
    consts = pconst.tile([P, _C_TOT], F32)
    nc.sync.dma_start(consts[:], dr["consts"][:])
    cvec = consts[:, _C_CVEC : _C_CVEC + 1]
    coef = consts[:, _C_COEF : _C_COEF + 4]
    maskd = consts[:, _C_MASK : _C_MASK + P]
    ident = consts[:, _C_IDENT : _C_IDENT + P]
    t0 = consts[:, _C_T0 : _C_T0 + S]
    rdb = consts[:, _C_RDB : _C_RDB + S]

    wqkv = pconst.tile([P, 3, 4, KD], F32)  # packed (Wq/8, Wk, Wv) lhsT chunks
    nc.sync.dma_start(wqkv[:], dr["wqkv"][:])
    wo = pconst.tile([KD, D], F32)
    nc.sync.dma_start(wo[:], dr["wo"][:])

    # ---- FIRE bias (transposed): biasT[:, jc, n] = bias^T[128*jc + p, 128*jc + n]
    biasT = pbias.tile([P, NJC, S], F32)
    for jc in range(NJC):
        W = S - P * jc
        num = ptmp.tile([P, S], F32, tag="tA")
        nc.vector.tensor_scalar(num[:, :W], t0[:, :W], 0.0, cvec, ALU.max, ALU.mult)
        lnv = ptmp.tile([P, S], F32, tag="tB")
        nc.scalar.activation(lnv[:, :W], num[:, :W], AF.Ln, bias=1.0, scale=1.0)
        r = ptmp.tile([P, S], F32, tag="tC")
        nc.vector.tensor_tensor(r[:, :W], lnv[:, :W], rdb[:, P * jc : P * jc + W], ALU.mult)
        q1 = ptmp.tile([P, S], F32, tag="tA")
        nc.vector.tensor_scalar(
            q1[:, :W], r[:, :W], coef[:, 3:4], coef[:, 1:2], ALU.mult, ALU.add
        )
        q0 = ptmp.tile([P, S], F32, tag="tB")
        nc.vector.tensor_scalar(
            q0[:, :W], r[:, :W], coef[:, 2:3], coef[:, 0:1], ALU.mult, ALU.add
        )
        r2 = ptmp.tile([P, S], F32, tag="tD")
        nc.vector.tensor_tensor(r2[:, :W], r[:, :W], r[:, :W], ALU.mult)
        t = ptmp.tile([P, S], F32, tag="tC")
        nc.vector.tensor_tensor(t[:, :W], r2[:, :W], q1[:, :W], ALU.mult)
        nc.vector.tensor_tensor(biasT[:, jc, :W], t[:, :W], q0[:, :W], ALU.add)
        # causal mask on the diagonal 128-block (j > i -> -30000)
        nc.vector.tensor_tensor(
            biasT[:, jc, 0:P], biasT[:, jc, 0:P], maskd, ALU.add
        )

    # ---- attention, one batch at a time
    for b in range(B):
        st = psrc.tile([P, 4, S], F32)
        for c in range(4):
            nc.sync.dma_start(st[:, c, :], dr["srcT"][b, P * c : P * (c + 1), :])

        # q^T, k^T, v^T : [KD, S] = W[KD, D] @ src^T (contraction over d-chunks)
        qkvT = []
        for w_i, tg in ((0, "qT"), (1, "kT"), (2, "vT")):
            dst = pqk.tile([KD, S], F32, tag=tg)
            for half in range(2):
                pp = ps_proj.tile([P, 512], F32, tag="pp")
                for c in range(4):
                    nc.tensor.matmul(
                        pp[:KD, :],
                        wqkv[:, w_i, c, :],
                        st[:, c, 512 * half : 512 * (half + 1)],
                        start=(c == 0),
                        stop=(c == 3),
                    )
                nc.vector.tensor_copy(dst[:, 512 * half : 512 * (half + 1)], pp[:KD, :])
            qkvT.append(dst)
        qT, kT, vT = qkvT

        # v' = [v natural | ones column] per key-block, via PE transpose
        vp = pvp.tile([P, NJC, KD + 1], F32)
        for jc in range(NJC):
            pt = ps_proj.tile([P, 512], F32, tag="pp")
            nc.tensor.transpose(
                pt[:, :KD], vT[:, P * jc : P * (jc + 1)], ident[:KD, :KD]
            )
            nc.vector.tensor_copy(vp[:, jc, :KD], pt[:, :KD])
            nc.vector.memset(vp[:, jc, KD : KD + 1], 1.0)

        # logits^T -> exp -> AV (triangular: i-window [128*jc, S))
        oT = ps_oT.tile([KD + 1, S], F32)
        for jc in range(NJC):
            W = S - P * jc
            at = pattn.tile([P, S], F32)
            for n0 in range(0, W, 512):
                nn = min(512, W - n0)
                lg = ps_log.tile([P, 512], F32, tag="lg")
                nc.tensor.matmul(
                    lg[:, :nn],
                    ident,
                    biasT[:, jc, n0 : n0 + nn],
                    start=True,
                    stop=False,
                    skip_group_check=True,
                )
                nc.tensor.matmul(
                    lg[:, :nn],
                    kT[:, P * jc : P * (jc + 1)],
                    qT[:, P * jc + n0 : P * jc + n0 + nn],
                    start=False,
                    stop=True,
                    skip_group_check=True,
                )
                nc.scalar.activation(at[:, n0 : n0 + nn], lg[:, :nn], AF.Exp)
            # accumulate into oT output chunks [0,512) and [512,1024)
            for oc in (0, 512):
                lo = max(oc, P * jc)
                hi = oc + 512
                if lo >= hi:
                    continue
                n0 = lo - P * jc
                nc.tensor.matmul(
                    oT[:, lo:hi],
                    vp[:, jc, :],
                    at[:, n0 : n0 + (hi - lo)],
                    start=(jc == 0),
                    stop=(jc == NJC - 1 or (oc == 0 and jc == 3)),
                    skip_group_check=True,
                )

        # row sums -> transpose to [tok-on-partition, tile] -> reciprocal (128-lane)
        sums_sb = prs.tile([1, S], F32, tag="sums")
        nc.scalar.copy(sums_sb[:], oT[KD : KD + 1, :])
        rsT_ps = ps_wo.tile([P, 512], F32, tag="po")
        for ti in range(NJC):
            nc.tensor.matmul(
                rsT_ps[:, ti : ti + 1],
                sums_sb[0:1, P * ti : P * (ti + 1)],
                ident[0:1, 0:1],
                start=True,
                stop=True,
                skip_group_check=True,
            )
        rsT = prs.tile([P, NJC], F32, tag="rsT")
        nc.vector.reciprocal(rsT[:], rsT_ps[:, :NJC])

        osb = posb.tile([KD, S], F32)
        nc.vector.tensor_copy(osb[:, 0:512], oT[:KD, 0:512])
        nc.vector.tensor_copy(osb[:, 512:S], oT[:KD, 512:S])

        # partial out = (o * rs) @ Wo_h^T ; rs folded into the PSUM->SBUF copy
        for ti in range(NJC):
            po = ps_wo.tile([P, 512], F32, tag="po")
            nc.tensor.matmul(
                po[:], osb[:, P * ti : P * (ti + 1)], wo[:], start=True, stop=True
            )
            ob = pout.tile([P, D], F32)
            if ti % 2 == 0:
                nc.scalar.activation(
                    ob[:], po[:], AF.Copy, bias=0.0, scale=rsT[:, ti : ti + 1]
                )
            else:
                nc.vector.tensor_scalar(
                    ob[:], po[:], rsT[:, ti : ti + 1], None, ALU.mult
                )
            nc.sync.dma_start(dr["out"][b, P * ti : P * (ti + 1), :], ob[:])


_NC_CACHE = {}


def _get_nc():
    if "nc" in _NC_CACHE:
        return _NC_CACHE["nc"]
    nc = bacc.Bacc("TRN2", target_bir_lowering=False, debug=False, num_devices=NCORES)
    dr = {
        "srcT": nc.dram_tensor("srcT", [B, D, S], F32, kind="ExternalInput"),
        "wqkv": nc.dram_tensor("wqkv", [P, 3, 4, KD], F32, kind="ExternalInput"),
        "wo": nc.dram_tensor("wo", [KD, D], F32, kind="ExternalInput"),
        "consts": nc.dram_tensor("consts", [P, _C_TOT], F32, kind="ExternalInput"),
        "out": nc.dram_tensor("out", [B, S, D], F32, kind="ExternalOutput"),
    }
    with tile.TileContext(nc) as tc:
        with ExitStack() as ctx:
            _build_kernel(ctx, tc, dr)
    nc.compile()
    _NC_CACHE["nc"] = nc
    return nc


_erf = np.frompyfunc(math.erf, 1, 1)


def _gelu64(x):
    return 0.5 * x * (1.0 + _erf(x).astype(np.float64))


def _host_prep(inputs):
    """Per-core input tensors (one head per core)."""
    src = np.ascontiguousarray(inputs["src"], dtype=np.float32)
    srcT = np.ascontiguousarray(src.transpose(0, 2, 1))  # [B, D, S]

    t0 = (
        np.arange(S, dtype=np.float32)[None, :]
        - np.arange(P, dtype=np.float32)[:, None]
    )
    maskd = np.where(
        np.arange(P)[:, None] > np.arange(P)[None, :], np.float32(MASK_NEG), 0.0
    ).astype(np.float32)
    identity = np.eye(P, dtype=np.float32)

    grid = np.linspace(0.0, 1.0, 4097)
    in_maps = []
    for h in range(H):
        c = float(np.logaddexp(0.0, np.float64(inputs["c_raw"][h])))
        L = float(inputs["L"][h])
        i = np.arange(S, dtype=np.float64)
        rd = (1.0 / np.log1p(c * np.maximum(L, i + 1.0))).astype(np.float32)
        rdb = np.broadcast_to(rd[None, :], (P, S))

        w1 = inputs["w1"][h].astype(np.float64)
        b1 = inputs["b1"][h].astype(np.float64)
        W2 = inputs["W2"][h].astype(np.float64)
        b2 = inputs["b2"][h].astype(np.float64)
        w3 = inputs["w3"][h].astype(np.float64)
        b3 = float(inputs["b3"][h])
        h1 = _gelu64(grid[:, None] * w1[None, :] + b1[None, :]).astype(np.float64)
        h2 = _gelu64(h1 @ W2.T + b2[None, :]).astype(np.float64)
        vals = h2 @ w3 + b3
        c3, c2, c1, c0 = np.polyfit(grid, vals, 3)

        consts = np.zeros((P, _C_TOT), np.float32)
        consts[:, _C_CVEC] = c
        consts[:, _C_COEF : _C_COEF + 4] = np.float32([c0, c1, c2, c3])
        consts[:, _C_MASK : _C_MASK + P] = maskd
        consts[:, _C_IDENT : _C_IDENT + P] = identity
        consts[:, _C_T0 : _C_T0 + S] = t0
        consts[:, _C_RDB : _C_RDB + S] = rdb

        # lhsT chunks: wqkv[p, w, ch, kd] = W[kd, 128*ch + p]  (Wq scaled by 1/8)
        wqkv = np.zeros((P, 3, 4, KD), np.float32)
        for w_i, (w_arr, scale) in enumerate(
            ((inputs["Wq"][h], 1.0 / 8.0), (inputs["Wk"][h], 1.0), (inputs["Wv"][h], 1.0))
        ):
            wt = (w_arr.astype(np.float64) * scale).astype(np.float32)  # [KD, D]
            wqkv[:, w_i, :, :] = wt.T.reshape(4, P, KD).transpose(1, 0, 2)

        wo = np.ascontiguousarray(
            inputs["Wo"][:, h * KD : (h + 1) * KD].T, dtype=np.float32
        )  # [KD, D]

        in_maps.append(
            {
                "srcT": srcT,
                "wqkv": wqkv,
                "wo": wo,
                "consts": consts,
            }
        )
    return in_maps


def run_on_device(inputs, **spmd_kwargs):
    """Compile (cached) + run; returns (per-core result dicts, BassKernelResults)."""
    nc = _get_nc()
    in_maps = _host_prep(inputs)
    res = run_bass_kernel_spmd(nc, in_maps, list(range(NCORES)), **spmd_kwargs)
    return res


def kernel(**inputs) -> np.ndarray:
    inputs = {k: np.asarray(v) for k, v in inputs.items()}
    res = run_on_device(inputs)
    out = np.zeros((B, S, D), np.float32)
    for h in range(H):
        out += res.results[h]["out"]
    return out


# revision 9
# speedup vs baseline: 2.2165x; 1.9730x over previous
"""FIRE self-attention TRN2 kernel.

Full inputs -> full output. Sharding: one attention head per NeuronCore
(8 heads / 8 cores, tensor parallel). Each core computes its head's FIRE
bias, QK^T logits, softmax, AV, and its head's slice of the output
projection; the host sums the 8 partial projections.

Key algorithmic points:
  * raw[i,j] = log1p(c*relu(i-j)) / log1p(c*max(L, i+1)) is always in
    [0, 1] (numerator <= denominator since i-j <= max(L, i+1)), and the
    per-head scalar MLP bias = f_theta(raw) is a smooth 1-D function, so
    it is evaluated as a cubic polynomial fitted on [0, 1] (fit error
    ~1e-7, far below fp32 matmul noise).
  * Everything runs in a transposed layout: logits^T[j, i] so that the
    softmax normalization axis (j) lands on PSUM partitions, attn^T is
    directly the stationary operand of the AV matmul, and a ones-column
    appended to V yields the softmax row sums for free.
  * The bias enters PSUM through an identity matmul so QK^T accumulates
    on top of it; the causal mask is -30000 added to the strict upper
    triangle (exp -> 0).
  * Softmax normalization (1/rowsum) is folded into the PSUM->SBUF copy
    of the output projection as a per-partition tensor_scalar multiply.
"""

import math
from contextlib import ExitStack

import numpy as np

import concourse.bacc as bacc
import concourse.bass as bass
import concourse.mybir as mybir
import concourse.tile as tile
from concourse.bass_utils import run_bass_kernel_spmd

F32 = mybir.dt.float32
F32R = mybir.dt.float32r  # tf32-like: 1 cyc/row on PE (fp32 is 4), ~1.5e-4 rel
AF = mybir.ActivationFunctionType
ALU = mybir.AluOpType

B, S, D, H, KD, HID = 8, 1024, 512, 8, 64, 32
P = 128
NJC = S // P  # 8 key-blocks of 128
NCORES = 8
MASK_NEG = -30000.0

# consts tensor column layout: cvec | coef(4) | maskd(128) | ident(128) | t0(S) | rdb(S)
_C_CVEC = 0
_C_COEF = 1
_C_MASK = 5
_C_IDENT = 133
_C_T0 = 261
_C_RDB = 261 + S
_C_TOT = 261 + 2 * S


def _build_kernel(ctx: ExitStack, tc: "tile.TileContext", dr):
    nc = tc.nc

    pconst = ctx.enter_context(tc.tile_pool(name="const", bufs=1))
    pbias = ctx.enter_context(tc.tile_pool(name="bias", bufs=1))
    ptmp = ctx.enter_context(tc.tile_pool(name="tmp", bufs=1))
    psrc = ctx.enter_context(tc.tile_pool(name="src", bufs=2))
    pqk = ctx.enter_context(tc.tile_pool(name="qk", bufs=2))
    pvp = ctx.enter_context(tc.tile_pool(name="vp", bufs=2))
    pattn = ctx.enter_context(tc.tile_pool(name="attn", bufs=3))
    posb = ctx.enter_context(tc.tile_pool(name="osb", bufs=2))
    prs = ctx.enter_context(tc.tile_pool(name="rs", bufs=2))
    pout = ctx.enter_context(tc.tile_pool(name="outst", bufs=3))

    # 4 PSUM pools x 2 banks = 8 banks, all double-buffered so batch b+1's
    # projections never wait on batch b's output-projection tail.
    ps_log = ctx.enter_context(
        tc.tile_pool(name="pslog", bufs=2, space=bass.MemorySpace.PSUM)
    )
    ps_oT = ctx.enter_context(
        tc.tile_pool(name="psoT", bufs=1, space=bass.MemorySpace.PSUM)
    )
    ps_proj = ctx.enter_context(
        tc.tile_pool(name="psproj", bufs=2, space=bass.MemorySpace.PSUM)
    )
    ps_wo = ctx.enter_context(
        tc.tile_pool(name="pswo", bufs=2, space=bass.MemorySpace.PSUM)
    )

    # ---- constants / weights into SBUF
    consts = pconst.tile([P, _C_TOT], F32)
    nc.sync.dma_start(consts[:], dr["consts"][:])
    cvec = consts[:, _C_CVEC : _C_CVEC + 1]
    coef = consts[:, _C_COEF : _C_COEF + 4]
    maskd = consts[:, _C_MASK : _C_MASK + P]
    ident = consts[:, _C_IDENT : _C_IDENT + P]
    t0 = consts[:, _C_T0 : _C_T0 + S]
    rdb = consts[:, _C_RDB : _C_RDB + S]

    wqkv = pconst.tile([P, 3, 4, KD], F32R)  # packed (Wq/8, Wk, Wv) lhsT chunks
    nc.sync.dma_start(wqkv[:], dr["wqkv"][:])
    wo = pconst.tile([KD, D], F32R)
    nc.sync.dma_start(wo[:], dr["wo"][:])
    identr = pconst.tile([P, P], F32R)
    nc.sync.dma_start(identr[:], dr["identr"][:])
    onesr = pconst.tile([P, 1], F32R)
    nc.sync.dma_start(onesr[:], dr["onesr"][:])

    # ---- FIRE bias (transposed): biasT[:, jc, n] = bias^T[128*jc + p, 128*jc + n]
    biasT = pbias.tile([P, NJC, S], F32R)
    for jc in range(NJC):
        W = S - P * jc
        num = ptmp.tile([P, S], F32, tag="tA")
        nc.vector.tensor_scalar(num[:, :W], t0[:, :W], 0.0, cvec, ALU.max, ALU.mult)
        lnv = ptmp.tile([P, S], F32, tag="tB")
        nc.scalar.activation(lnv[:, :W], num[:, :W], AF.Ln, bias=1.0, scale=1.0)
        r = ptmp.tile([P, S], F32, tag="tC")
        nc.vector.tensor_tensor(r[:, :W], lnv[:, :W], rdb[:, P * jc : P * jc + W], ALU.mult)
        q1 = ptmp.tile([P, S], F32, tag="tA")
        nc.vector.tensor_scalar(
            q1[:, :W], r[:, :W], coef[:, 3:4], coef[:, 1:2], ALU.mult, ALU.add
        )
        q0 = ptmp.tile([P, S], F32, tag="tB")
        nc.vector.tensor_scalar(
            q0[:, :W], r[:, :W], coef[:, 2:3], coef[:, 0:1], ALU.mult, ALU.add
        )
        r2 = ptmp.tile([P, S], F32, tag="tD")
        nc.vector.tensor_tensor(r2[:, :W], r[:, :W], r[:, :W], ALU.mult)
        t = ptmp.tile([P, S], F32, tag="tC")
        nc.vector.tensor_tensor(t[:, :W], r2[:, :W], q1[:, :W], ALU.mult)
        nc.vector.tensor_tensor(biasT[:, jc, :W], t[:, :W], q0[:, :W], ALU.add)
        # causal mask on the diagonal 128-block (j > i -> -30000)
        nc.vector.tensor_tensor(
            biasT[:, jc, 0:P], biasT[:, jc, 0:P], maskd, ALU.add
        )

    # ---- attention, one batch at a time
    for b in range(B):
        st = psrc.tile([P, 4, S], F32R)
        for c in range(4):
            nc.sync.dma_start(st[:, c, :], dr["srcT"][b, P * c : P * (c + 1), :])

        # q^T, k^T, v^T : [KD, S] = W[KD, D] @ src^T (contraction over d-chunks)
        qkvT = []
        for w_i, tg in ((0, "qT"), (1, "kT"), (2, "vT")):
            dst = pqk.tile([KD, S], F32R, tag=tg)
            for half in range(2):
                pp = ps_proj.tile([P, 512], F32, tag="pp")
                for c in range(4):
                    nc.tensor.matmul(
                        pp[:KD, :],
                        wqkv[:, w_i, c, :],
                        st[:, c, 512 * half : 512 * (half + 1)],
                        start=(c == 0),
                        stop=(c == 3),
                    )
                nc.vector.tensor_copy(dst[:, 512 * half : 512 * (half + 1)], pp[:KD, :])
            qkvT.append(dst)
        qT, kT, vT = qkvT

        # v' = [v natural | ones column] per key-block, via PE transpose
        vp = pvp.tile([P, NJC, KD + 1], F32R)
        for jc in range(NJC):
            pt = ps_proj.tile([P, 512], F32R, tag="pp")
            nc.tensor.transpose(
                pt[:, :KD], vT[:, P * jc : P * (jc + 1)], identr[:KD, :KD]
            )
            nc.vector.tensor_copy(vp[:, jc, :KD], pt[:, :KD])
            nc.vector.tensor_copy(vp[:, jc, KD : KD + 1], onesr[:])

        # logits^T -> exp -> AV (triangular: i-window [128*jc, S))
        oT = ps_oT.tile([KD + 1, S], F32)
        for jc in range(NJC):
            W = S - P * jc
            at = pattn.tile([P, S], F32R)
            for n0 in range(0, W, 512):
                nn = min(512, W - n0)
                lg = ps_log.tile([P, 512], F32, tag="lg")
                nc.tensor.matmul(
                    lg[:, :nn],
                    identr,
                    biasT[:, jc, n0 : n0 + nn],
                    start=True,
                    stop=False,
                    skip_group_check=True,
                )
                nc.tensor.matmul(
                    lg[:, :nn],
                    kT[:, P * jc : P * (jc + 1)],
                    qT[:, P * jc + n0 : P * jc + n0 + nn],
                    start=False,
                    stop=True,
                    skip_group_check=True,
                )
                nc.scalar.activation(at[:, n0 : n0 + nn], lg[:, :nn], AF.Exp)
            # accumulate into oT output chunks [0,512) and [512,1024)
            for oc in (0, 512):
                lo = max(oc, P * jc)
                hi = oc + 512
                if lo >= hi:
                    continue
                n0 = lo - P * jc
                nc.tensor.matmul(
                    oT[:, lo:hi],
                    vp[:, jc, :],
                    at[:, n0 : n0 + (hi - lo)],
                    start=(jc == 0),
                    stop=(jc == NJC - 1 or (oc == 0 and jc == 3)),
                    skip_group_check=True,
                )

        # row sums -> transpose to [tok-on-partition, tile] -> reciprocal (128-lane)
        sums_sb = prs.tile([1, S], F32, tag="sums")
        nc.scalar.copy(sums_sb[:], oT[KD : KD + 1, :])
        rsT_ps = ps_wo.tile([P, 512], F32, tag="po")
        for ti in range(NJC):
            nc.tensor.matmul(
                rsT_ps[:, ti : ti + 1],
                sums_sb[0:1, P * ti : P * (ti + 1)],
                ident[0:1, 0:1],
                start=True,
                stop=True,
                skip_group_check=True,
            )
        rsT = prs.tile([P, NJC], F32, tag="rsT")
        nc.vector.reciprocal(rsT[:], rsT_ps[:, :NJC])

        osb = posb.tile([KD, S], F32R)
        nc.vector.tensor_copy(osb[:, 0:512], oT[:KD, 0:512])
        nc.vector.tensor_copy(osb[:, 512:S], oT[:KD, 512:S])

        # partial out = (o * rs) @ Wo_h^T ; rs folded into the PSUM->SBUF copy
        for ti in range(NJC):
            po = ps_wo.tile([P, 512], F32, tag="po")
            nc.tensor.matmul(
                po[:], osb[:, P * ti : P * (ti + 1)], wo[:], start=True, stop=True
            )
            ob = pout.tile([P, D], F32)
            if ti % 2 == 0:
                nc.scalar.activation(
                    ob[:], po[:], AF.Copy, bias=0.0, scale=rsT[:, ti : ti + 1]
                )
            else:
                nc.vector.tensor_scalar(
                    ob[:], po[:], rsT[:, ti : ti + 1], None, ALU.mult
                )
            nc.sync.dma_start(dr["out"][b, P * ti : P * (ti + 1), :], ob[:])


_NC_CACHE = {}


def _get_nc():
    if "nc" in _NC_CACHE:
        return _NC_CACHE["nc"]
    nc = bacc.Bacc("TRN2", target_bir_lowering=False, debug=False, num_devices=NCORES)
    dr = {
        "srcT": nc.dram_tensor("srcT", [B, D, S], F32R, kind="ExternalInput"),
        "wqkv": nc.dram_tensor("wqkv", [P, 3, 4, KD], F32R, kind="ExternalInput"),
        "wo": nc.dram_tensor("wo", [KD, D], F32R, kind="ExternalInput"),
        "identr": nc.dram_tensor("identr", [P, P], F32R, kind="ExternalInput"),
        "onesr": nc.dram_tensor("onesr", [P, 1], F32R, kind="ExternalInput"),
        "consts": nc.dram_tensor("consts", [P, _C_TOT], F32, kind="ExternalInput"),
        "out": nc.dram_tensor("out", [B, S, D], F32, kind="ExternalOutput"),
    }
    with tile.TileContext(nc) as tc:
        with ExitStack() as ctx:
            _build_kernel(ctx, tc, dr)
    nc.compile()
    _NC_CACHE["nc"] = nc
    return nc


_erf = np.frompyfunc(math.erf, 1, 1)


def _gelu64(x):
    return 0.5 * x * (1.0 + _erf(x).astype(np.float64))


def _host_prep(inputs):
    """Per-core input tensors (one head per core)."""
    src = np.ascontiguousarray(inputs["src"], dtype=np.float32)
    srcT = np.ascontiguousarray(src.transpose(0, 2, 1))  # [B, D, S]

    t0 = (
        np.arange(S, dtype=np.float32)[None, :]
        - np.arange(P, dtype=np.float32)[:, None]
    )
    maskd = np.where(
        np.arange(P)[:, None] > np.arange(P)[None, :], np.float32(MASK_NEG), 0.0
    ).astype(np.float32)
    identity = np.eye(P, dtype=np.float32)

    grid = np.linspace(0.0, 1.0, 4097)
    in_maps = []
    for h in range(H):
        c = float(np.logaddexp(0.0, np.float64(inputs["c_raw"][h])))
        L = float(inputs["L"][h])
        i = np.arange(S, dtype=np.float64)
        rd = (1.0 / np.log1p(c * np.maximum(L, i + 1.0))).astype(np.float32)
        rdb = np.broadcast_to(rd[None, :], (P, S))

        w1 = inputs["w1"][h].astype(np.float64)
        b1 = inputs["b1"][h].astype(np.float64)
        W2 = inputs["W2"][h].astype(np.float64)
        b2 = inputs["b2"][h].astype(np.float64)
        w3 = inputs["w3"][h].astype(np.float64)
        b3 = float(inputs["b3"][h])
        h1 = _gelu64(grid[:, None] * w1[None, :] + b1[None, :]).astype(np.float64)
        h2 = _gelu64(h1 @ W2.T + b2[None, :]).astype(np.float64)
        vals = h2 @ w3 + b3
        c3, c2, c1, c0 = np.polyfit(grid, vals, 3)

        consts = np.zeros((P, _C_TOT), np.float32)
        consts[:, _C_CVEC] = c
        consts[:, _C_COEF : _C_COEF + 4] = np.float32([c0, c1, c2, c3])
        consts[:, _C_MASK : _C_MASK + P] = maskd
        consts[:, _C_IDENT : _C_IDENT + P] = identity
        consts[:, _C_T0 : _C_T0 + S] = t0
        consts[:, _C_RDB : _C_RDB + S] = rdb

        # lhsT chunks: wqkv[p, w, ch, kd] = W[kd, 128*ch + p]  (Wq scaled by 1/8)
        wqkv = np.zeros((P, 3, 4, KD), np.float32)
        for w_i, (w_arr, scale) in enumerate(
            ((inputs["Wq"][h], 1.0 / 8.0), (inputs["Wk"][h], 1.0), (inputs["Wv"][h], 1.0))
        ):
            wt = (w_arr.astype(np.float64) * scale).astype(np.float32)  # [KD, D]
            wqkv[:, w_i, :, :] = wt.T.reshape(4, P, KD).transpose(1, 0, 2)

        wo = np.ascontiguousarray(
            inputs["Wo"][:, h * KD : (h + 1) * KD].T, dtype=np.float32
        )  # [KD, D]

        in_maps.append(
            {
                "identr": identity,
                "onesr": np.ones((P, 1), np.float32),
                "srcT": srcT,
                "wqkv": wqkv,
                "wo": wo,
                "consts": consts,
            }
        )
    return in_maps


def run_on_device(inputs, **spmd_kwargs):
    """Compile (cached) + run; returns (per-core result dicts, BassKernelResults)."""
    nc = _get_nc()
    in_maps = _host_prep(inputs)
    res = run_bass_kernel_spmd(nc, in_maps, list(range(NCORES)), **spmd_kwargs)
    return res


def kernel(**inputs) -> np.ndarray:
    inputs = {k: np.asarray(v) for k, v in inputs.items()}
    res = run_on_device(inputs)
    out = np.zeros((B, S, D), np.float32)
    for h in range(H):
        out += res.results[h]["out"]
    return out
